# revision 17
# baseline (speedup 1.0000x reference)
"""Trainium2 Bass kernel for 4-head spatial self-attention (nn_Attention).

Reference computation (shapes hardcoded):
  x [4, 256, 64, 64] --1x1conv--> qkv [4, 384, 64, 64]
  per (batch, head): sim = (q*scale)^T k over c_head=32, softmax over j,
  out = attn @ v^T, then 1x1 out-projection back to 256 channels.

Sharding: 8 cores = 4 batches x 2 spatial halves (i-halves of 2048 tokens).
Each core computes k,v for its full batch and q for its i-half, producing a
complete [256, 2048] output slice; the host just concatenates. All cores run
an identical program (only the input data differs).

Per-core kernel strategy:
  - sim is computed TRANSPOSED (j on partitions) via k^T q so the PV matmul
    needs no transpose; softmax max-subtraction is skipped (logits are ~N(0,1)
    for this problem, exp is safe in fp32, softmax is shift-invariant).
  - exp runs on the scalar engine as [128, 1024] PSUM->SBUF activations with
    bf16 output (fp32 output runs 3.5x slower); ACT is the bottleneck engine
    (~2.0us per j-tile) and everything else must hide underneath it.
  - QK uses float32r (1 cycle/col vs 4 for fp32 on the PE; the q/k psum->sbuf
    copies are the required "rounded" producers), PV runs in bf16.
  - The softmax denominator is fused into PV as a ones-column of vT (M=33
    matmuls, two heads col-packed per PSUM bank) instead of separate
    ones-vector matmuls: PSUM bank rows [0:32]=even head, row 32 = its
    denominator, [64:96]=odd head, row 96 = its denominator. The head
    interleave is undone for free by zero-padded split output-projection
    weights built on the host.
  - k / vT projections are streamed just ahead of their first use inside the
    first i-block so exp starts as early as possible.
"""

import numpy as np

HEADS = 4
C_HEAD = 32
C_IN = 256
C_HID = 128
B = 4
NJ = 4096  # full token count (64*64)
NI = 2048  # per-core i-half
IB = 512  # i-block (PSUM bank width in fp32)
NJT = NJ // 128  # 32 j-tiles
NIB = NI // IB  # 4 i-blocks
P = 128

_STATE = {}


def _build_program(reps=1):
    import concourse.bacc as bacc
    import concourse.tile as tile
    from concourse import mybir

    F32 = mybir.dt.float32
    BF16 = mybir.dt.bfloat16

    nc = bacc.Bacc(None, target_bir_lowering=False)

    xkv = nc.declare_dram_parameter("xkv", [C_IN, NJ], F32, isOutput=False)
    xq = nc.declare_dram_parameter("xq", [C_IN, NI], F32, isOutput=False)
    wq = nc.declare_dram_parameter("wq_t", [C_IN, C_HID], F32, isOutput=False)
    wk = nc.declare_dram_parameter("wk_t", [C_IN, C_HID], F32, isOutput=False)
    wv = nc.declare_dram_parameter("wv_t", [C_IN, C_HID], BF16, isOutput=False)
    woa = nc.declare_dram_parameter("woa_t", [C_HID, C_IN], F32, isOutput=False)
    wob = nc.declare_dram_parameter("wob_t", [C_HID, C_IN], F32, isOutput=False)
    bo = nc.declare_dram_parameter("bo", [2, P], F32, isOutput=False)
    out = nc.declare_dram_parameter("out", [C_IN, NI], F32, isOutput=True)

    with tile.TileContext(nc) as tc:
        with (
            tc.tile_pool(name="consts", bufs=1) as consts,
            tc.tile_pool(name="xpool", bufs=1) as xpool,
            tc.tile_pool(name="qkv", bufs=1) as qkv,
            tc.tile_pool(name="epool", bufs=4) as epool,
            tc.tile_pool(name="misc", bufs=2) as misc,
            tc.tile_pool(name="opool", bufs=4) as opool,
            tc.tile_pool(name="psim", bufs=3, space="PSUM") as psim,
            tc.tile_pool(name="ppv", bufs=1, space="PSUM") as ppv,
        ):
            # --- constants / weights (loaded once) ---
            wq_t = consts.tile([P, 2, C_HID], F32)
            nc.sync.dma_start(out=wq_t, in_=wq[:].rearrange("(t p) m -> p t m", p=P))
            wk_t = consts.tile([P, 2, C_HID], F32)
            nc.sync.dma_start(out=wk_t, in_=wk[:].rearrange("(t p) m -> p t m", p=P))
            wv_t = consts.tile([P, 2, C_HID], BF16)
            nc.sync.dma_start(out=wv_t, in_=wv[:].rearrange("(t p) m -> p t m", p=P))
            woa_t = consts.tile([P, C_IN], F32)
            nc.sync.dma_start(out=woa_t, in_=woa[:])
            wob_t = consts.tile([P, C_IN], F32)
            nc.sync.dma_start(out=wob_t, in_=wob[:])
            bo_t = consts.tile([P, 2], F32)
            nc.sync.dma_start(out=bo_t, in_=bo[:].rearrange("t p -> p t"))
            ones_row = consts.tile([P, C_HEAD], F32)
            nc.vector.memset(ones_row, 1.0)

            env = dict(
                xkv=xkv, xq=xq, out=out,
                wq_t=wq_t, wk_t=wk_t, wv_t=wv_t, woa_t=woa_t, wob_t=wob_t,
                bo_t=bo_t, ones_row=ones_row,
                xpool=xpool, qkv=qkv, epool=epool, misc=misc, opool=opool,
                psim=psim, ppv=ppv,
            )
            if reps == 1:
                _emit_body(nc, tc, mybir, env)
            else:
                with tc.For_i(0, reps, 1):
                    _emit_body(nc, tc, mybir, env)

    nc.compile()
    return nc


def _emit_body(nc, tc, mybir, env):
    F32 = mybir.dt.float32
    BF16 = mybir.dt.bfloat16
    F32R = mybir.dt.float32r
    EXP = mybir.ActivationFunctionType.Exp

    xkv, xq, out = env["xkv"], env["xq"], env["out"]
    wq_t, wk_t, wv_t = env["wq_t"], env["wk_t"], env["wv_t"]
    woa_t, wob_t, bo_t = env["woa_t"], env["wob_t"], env["bo_t"]
    ones_row = env["ones_row"]
    xpool, qkv, epool, misc, opool = (
        env["xpool"], env["qkv"], env["epool"], env["misc"], env["opool"],
    )
    psim, ppv = env["psim"], env["ppv"]

    xq_t = xpool.tile([P, 2, NI], F32)
    nc.sync.dma_start(out=xq_t, in_=xq[:].rearrange("(t p) n -> p t n", p=P))
    xkv_t = xpool.tile([P, 2, NJ], F32)
    nc.sync.dma_start(out=xkv_t, in_=xkv[:].rearrange("(t p) n -> p t n", p=P))

    xkv_bf = qkv.tile([P, 2, NJ], BF16)
    nc.vector.tensor_copy(xkv_bf, xkv_t)
    q_t = qkv.tile([P, NI], F32R)
    k_t = qkv.tile([P, NJ], F32R)
    # vT layout: [j-part, j-tile, head, 34]; cols 0:32 = v^T, col 32 = ones
    # (fused softmax denominator), col 33 = padding.
    vT_t = qkv.tile([P, NJT, HEADS, 34], BF16)
    nc.vector.memset(vT_t[:, :, :, 32:34], 1.0)

    # --- projections ---
    # q[c_hid, i] = wq_t.T @ xq ; softmax scale is folded into wq_t host-side
    for c0 in range(0, NI, IB):
        pq = psim.tile([P, IB], F32, tag="sim")
        for t in range(2):
            nc.tensor.matmul(
                pq, wq_t[:, t, :], xq_t[:, t, c0 : c0 + IB],
                start=(t == 0), stop=(t == 1),
            )
        nc.vector.tensor_copy(q_t[:, c0 : c0 + IB], pq)

    def emit_k_chunk(c0):
        pk = psim.tile([P, IB], F32, tag="sim")
        for t in range(2):
            nc.tensor.matmul(
                pk, wk_t[:, t, :], xkv_t[:, t, c0 : c0 + IB],
                start=(t == 0), stop=(t == 1),
            )
        nc.vector.tensor_copy(k_t[:, c0 : c0 + IB], pk)

    # vT[j, c_hid] = x_tile.T @ wv_t (x stationary, bf16)
    def emit_vt(jt):
        pv_ = psim.tile([P, C_HID], F32, tag="sim")
        for t in range(2):
            nc.tensor.matmul(
                pv_, xkv_bf[:, t, jt * P : (jt + 1) * P], wv_t[:, t, :],
                start=(t == 0), stop=(t == 1),
            )
        nc.vector.tensor_copy(
            vT_t[:, jt, :, 0:32],
            pv_[:].rearrange("p (h c) -> p h c", h=HEADS),
        )

    # --- attention ---
    for ib in range(NIB):
        isl = slice(ib * IB, (ib + 1) * IB)
        pv_a = ppv.tile([P, IB], F32, tag="pv0")
        pv_b = ppv.tile([P, IB], F32, tag="pv1")
        pv_ps = [pv_a, pv_b]
        for jt in range(NJT):
            jsl = slice(jt * P, (jt + 1) * P)
            if ib == 0:
                # stream the k / vT projections just ahead of first use so
                # the scalar engine starts exp-ing as early as possible
                if jt % 4 == 0:
                    emit_k_chunk(jt * P)
                emit_vt(jt)
            for pair in range(2):
                sim = psim.tile([P, 2, IB], F32, tag="sim")
                for hh in range(2):
                    h = pair * 2 + hh
                    hsl = slice(h * C_HEAD, (h + 1) * C_HEAD)
                    nc.tensor.matmul(
                        sim[:, hh, :], k_t[hsl, jsl], q_t[hsl, isl],
                        start=True, stop=True,
                        tile_position=(h * C_HEAD, 0),
                    )
                e_t = epool.tile([P, 2, IB], BF16, tag="e")
                nc.scalar.activation(e_t, sim, EXP)
                for hh in range(2):
                    h = pair * 2 + hh
                    nc.tensor.matmul(
                        pv_ps[pair][64 * hh : 64 * hh + 33, :],
                        vT_t[:, jt, h, 0:33], e_t[:, hh, :],
                        start=(jt == 0), stop=(jt == NJT - 1),
                        tile_position=(0, 64 * hh),
                    )
        # normalization: head rows scale by 1/denominator per column
        ao = []
        for pair in range(2):
            recip = misc.tile([P, IB], F32, tag=f"recip{pair}")
            nc.vector.reciprocal(recip[0:97, :], pv_ps[pair][0:97, :])
            bc_ps = psim.tile([P, IB], F32, tag="sim")
            for hh in range(2):
                nc.tensor.matmul(
                    bc_ps[64 * hh : 64 * hh + 32, :],
                    ones_row[32 + 64 * hh : 33 + 64 * hh, :],
                    recip[32 + 64 * hh : 33 + 64 * hh, :],
                    start=True, stop=True,
                    tile_position=(32 + 64 * hh, 64 * hh),
                )
            bc_sb = misc.tile([P, IB], F32, tag=f"bcsb{pair}")
            nc.vector.tensor_copy(bc_sb, bc_ps)
            ao_t = misc.tile([P, IB], F32, tag=f"ao{pair}")
            for hh in range(2):
                rs = slice(64 * hh, 64 * hh + 32)
                zs = slice(64 * hh + 32, 64 * hh + 64)
                nc.vector.tensor_mul(ao_t[rs, :], pv_ps[pair][rs, :], bc_sb[rs, :])
                nc.vector.memset(ao_t[zs, :], 0.0)
            ao.append(ao_t)
        # output projection (zero-padded split weights undo the head
        # interleave) + bias
        for ot in range(2):
            pr_ps = ppv.tile([P, IB], F32, tag=f"pv{ot}")
            osl = slice(ot * P, (ot + 1) * P)
            nc.tensor.matmul(pr_ps, woa_t[:, osl], ao[0], start=True, stop=False)
            nc.tensor.matmul(pr_ps, wob_t[:, osl], ao[1], start=False, stop=True)
            o_t = opool.tile([P, IB], F32, tag="o")
            nc.vector.tensor_scalar_add(o_t, pr_ps, bo_t[:, ot : ot + 1])
            nc.sync.dma_start(
                out=out[:].rearrange("(t p) n -> p t n", p=P)[:, ot, isl],
                in_=o_t,
            )


def _get_nc(reps=1):
    key = ("nc", reps)
    if key not in _STATE:
        _STATE[key] = _build_program(reps)
    return _STATE[key]


def _to_bf16(a):
    import ml_dtypes

    return np.ascontiguousarray(a).astype(ml_dtypes.bfloat16)


def _make_in_maps(x, w_qkv, w_out, b_out):
    x = np.ascontiguousarray(x, dtype=np.float32)
    w_qkv = np.asarray(w_qkv, dtype=np.float32)
    w_out = np.asarray(w_out, dtype=np.float32)
    b_out = np.asarray(b_out, dtype=np.float32)
    scale = np.float32(C_HEAD**-0.5)
    wo_t = w_out.T  # [c_hid, c_in]
    woa = np.zeros((C_HID, C_IN), np.float32)
    wob = np.zeros((C_HID, C_IN), np.float32)
    woa[0:32] = wo_t[0:32]  # head 0
    woa[64:96] = wo_t[32:64]  # head 1
    wob[0:32] = wo_t[64:96]  # head 2
    wob[64:96] = wo_t[96:128]  # head 3
    shared = {
        "wq_t": np.ascontiguousarray((w_qkv[0:C_HID] * scale).T),
        "wk_t": np.ascontiguousarray(w_qkv[C_HID : 2 * C_HID].T),
        "wv_t": _to_bf16(w_qkv[2 * C_HID : 3 * C_HID].T),
        "woa_t": woa,
        "wob_t": wob,
        "bo": np.ascontiguousarray(b_out.reshape(2, P)),
    }
    in_maps = []
    for c in range(8):
        b, half = divmod(c, 2)
        xkv = np.ascontiguousarray(x[b].reshape(C_IN, NJ))
        xq = np.ascontiguousarray(xkv[:, half * NI : (half + 1) * NI])
        in_maps.append({"xkv": xkv, "xq": xq, **shared})
    return in_maps


def _assemble(results):
    out = np.empty((B, C_IN, NJ), np.float32)
    for c in range(8):
        b, half = divmod(c, 2)
        out[b][:, half * NI : (half + 1) * NI] = results[c]["out"]
    return out.reshape(B, C_IN, 64, 64)


def _run(in_maps, reps=1, **kwargs):
    from concourse.bass_utils import run_bass_kernel_spmd

    return run_bass_kernel_spmd(
        _get_nc(reps), in_maps, core_ids=list(range(8)), **kwargs
    )


def kernel(x, w_qkv, w_out, b_out):
    res = _run(_make_in_maps(x, w_qkv, w_out, b_out))
    return _assemble(res.results)


# revision 18
# speedup vs baseline: 1.3584x; 1.3584x over previous
"""Trainium2 Bass kernel for 4-head spatial self-attention (nn_Attention).

Reference computation (shapes hardcoded):
  x [4, 256, 64, 64] --1x1conv--> qkv [4, 384, 64, 64]
  per (batch, head): sim = (q*scale)^T k over c_head=32, softmax over j,
  out = attn @ v^T, then 1x1 out-projection back to 256 channels.

Sharding: 8 cores = 4 batches x 2 spatial halves (i-halves of 2048 tokens).
Each core computes k,v for its full batch and q for its i-half, producing a
complete [256, 2048] output slice; the host just concatenates. All cores run
an identical program (only the input data differs).

Per-core kernel strategy:
  - sim is computed TRANSPOSED (j on partitions) via k^T q so the PV matmul
    needs no transpose; softmax max-subtraction is skipped (logits are ~N(0,1)
    for this problem, exp is safe in fp32, softmax is shift-invariant).
  - exp runs on the scalar engine as [128, 1024] PSUM->SBUF activations with
    bf16 output (fp32 output runs 3.5x slower); ACT is the bottleneck engine
    (~2.0us per j-tile) and everything else must hide underneath it.
  - QK uses float32r (1 cycle/col vs 4 for fp32 on the PE; the q/k psum->sbuf
    copies are the required "rounded" producers), PV runs in bf16.
  - The softmax denominator is fused into PV as a ones-column of vT (M=33
    matmuls, two heads col-packed per PSUM bank) instead of separate
    ones-vector matmuls: PSUM bank rows [0:32]=even head, row 32 = its
    denominator, [64:96]=odd head, row 96 = its denominator. The head
    interleave is undone for free by zero-padded split output-projection
    weights built on the host.
  - k / vT projections are streamed just ahead of their first use inside the
    first i-block so exp starts as early as possible.
"""

import numpy as np

HEADS = 4
C_HEAD = 32
C_IN = 256
C_HID = 128
B = 4
NJ = 4096  # full token count (64*64)
NI = 2048  # per-core i-half
IB = 512  # i-block (PSUM bank width in fp32)
NJT = NJ // 128  # 32 j-tiles
NIB = NI // IB  # 4 i-blocks
P = 128

_STATE = {}


def _build_program(reps=1):
    import concourse.bacc as bacc
    import concourse.tile as tile
    from concourse import mybir

    F32 = mybir.dt.float32
    BF16 = mybir.dt.bfloat16

    nc = bacc.Bacc(None, target_bir_lowering=False)

    xkv = nc.declare_dram_parameter("xkv", [C_IN, NJ], F32, isOutput=False)
    xq = nc.declare_dram_parameter("xq", [C_IN, NI], F32, isOutput=False)
    wq = nc.declare_dram_parameter("wq_t", [C_IN, C_HID], F32, isOutput=False)
    wk = nc.declare_dram_parameter("wk_t", [C_IN, C_HID], F32, isOutput=False)
    wv = nc.declare_dram_parameter("wv_t", [C_IN, C_HID], BF16, isOutput=False)
    woa = nc.declare_dram_parameter("woa_t", [C_HID, C_IN], F32, isOutput=False)
    wob = nc.declare_dram_parameter("wob_t", [C_HID, C_IN], F32, isOutput=False)
    bo = nc.declare_dram_parameter("bo", [2, P], F32, isOutput=False)
    out = nc.declare_dram_parameter("out", [C_IN, NI], F32, isOutput=True)

    with tile.TileContext(nc) as tc:
        with (
            tc.tile_pool(name="consts", bufs=1) as consts,
            tc.tile_pool(name="xpool", bufs=1) as xpool,
            tc.tile_pool(name="qkv", bufs=1) as qkv,
            tc.tile_pool(name="epool", bufs=4) as epool,
            tc.tile_pool(name="misc", bufs=2) as misc,
            tc.tile_pool(name="opool", bufs=4) as opool,
            tc.tile_pool(name="psim", bufs=3, space="PSUM") as psim,
            tc.tile_pool(name="ppv", bufs=1, space="PSUM") as ppv,
        ):
            # --- constants / weights (loaded once) ---
            wq_t = consts.tile([P, 2, C_HID], F32)
            nc.sync.dma_start(out=wq_t, in_=wq[:].rearrange("(t p) m -> p t m", p=P))
            wk_t = consts.tile([P, 2, C_HID], F32)
            nc.sync.dma_start(out=wk_t, in_=wk[:].rearrange("(t p) m -> p t m", p=P))
            wv_t = consts.tile([P, 2, C_HID], BF16)
            nc.sync.dma_start(out=wv_t, in_=wv[:].rearrange("(t p) m -> p t m", p=P))
            woa_t = consts.tile([P, C_IN], F32)
            nc.sync.dma_start(out=woa_t, in_=woa[:])
            wob_t = consts.tile([P, C_IN], F32)
            nc.sync.dma_start(out=wob_t, in_=wob[:])
            bo_t = consts.tile([P, 2], F32)
            nc.sync.dma_start(out=bo_t, in_=bo[:].rearrange("t p -> p t"))
            ones_row = consts.tile([P, C_HEAD], F32)
            nc.vector.memset(ones_row, 1.0)
            wq_r = consts.tile([P, 2, C_HID], mybir.dt.float32r)
            nc.vector.tensor_copy(wq_r, wq_t)
            wk_r = consts.tile([P, 2, C_HID], mybir.dt.float32r)
            nc.vector.tensor_copy(wk_r, wk_t)

            env = dict(
                xkv=xkv, xq=xq, out=out,
                wq_t=wq_r, wk_t=wk_r, wv_t=wv_t, woa_t=woa_t, wob_t=wob_t,
                bo_t=bo_t, ones_row=ones_row,
                xpool=xpool, qkv=qkv, epool=epool, misc=misc, opool=opool,
                psim=psim, ppv=ppv,
            )
            if reps == 1:
                _emit_body(nc, tc, mybir, env)
            else:
                with tc.For_i(0, reps, 1):
                    _emit_body(nc, tc, mybir, env)

    nc.compile()
    return nc


def _emit_body(nc, tc, mybir, env):
    F32 = mybir.dt.float32
    BF16 = mybir.dt.bfloat16
    F32R = mybir.dt.float32r
    EXP = mybir.ActivationFunctionType.Exp

    xkv, xq, out = env["xkv"], env["xq"], env["out"]
    wq_t, wk_t, wv_t = env["wq_t"], env["wk_t"], env["wv_t"]
    woa_t, wob_t, bo_t = env["woa_t"], env["wob_t"], env["bo_t"]
    ones_row = env["ones_row"]
    xpool, qkv, epool, misc, opool = (
        env["xpool"], env["qkv"], env["epool"], env["misc"], env["opool"],
    )
    psim, ppv = env["psim"], env["ppv"]

    xq_t = xpool.tile([P, 2, NI], F32)
    nc.sync.dma_start(out=xq_t, in_=xq[:].rearrange("(t p) n -> p t n", p=P))
    xkv_t = xpool.tile([P, 2, NJ], F32)
    nc.sync.dma_start(out=xkv_t, in_=xkv[:].rearrange("(t p) n -> p t n", p=P))

    xkv_bf = qkv.tile([P, 2, NJ], BF16)
    nc.vector.tensor_copy(xkv_bf, xkv_t)
    xq_r = qkv.tile([P, 2, NI], F32R)
    nc.vector.tensor_copy(xq_r, xq_t)
    xkv_r = qkv.tile([P, 2, NJ], F32R)
    nc.vector.tensor_copy(xkv_r, xkv_t)
    q_t = qkv.tile([P, NI], F32R)
    k_t = qkv.tile([P, NJ], F32R)
    # vT layout: [j-part, j-tile, head, 34]; cols 0:32 = v^T, col 32 = ones
    # (fused softmax denominator), col 33 = padding.
    vT_t = qkv.tile([P, NJT, HEADS, 34], BF16)
    nc.vector.memset(vT_t[:, :, :, 32:34], 1.0)

    # --- projections ---
    # q[c_hid, i] = wq_t.T @ xq ; softmax scale is folded into wq_t host-side
    for c0 in range(0, NI, IB):
        pq = psim.tile([P, IB], F32, tag="sim")
        for t in range(2):
            nc.tensor.matmul(
                pq, wq_t[:, t, :], xq_r[:, t, c0 : c0 + IB],
                start=(t == 0), stop=(t == 1),
            )
        nc.vector.tensor_copy(q_t[:, c0 : c0 + IB], pq)

    def emit_k_chunk(c0):
        pk = psim.tile([P, IB], F32, tag="sim")
        for t in range(2):
            nc.tensor.matmul(
                pk, wk_t[:, t, :], xkv_r[:, t, c0 : c0 + IB],
                start=(t == 0), stop=(t == 1),
            )
        nc.vector.tensor_copy(k_t[:, c0 : c0 + IB], pk)

    # vT[j, c_hid] = x_tile.T @ wv_t (x stationary, bf16)
    def emit_vt(jt):
        pv_ = psim.tile([P, C_HID], F32, tag="sim")
        for t in range(2):
            nc.tensor.matmul(
                pv_, xkv_bf[:, t, jt * P : (jt + 1) * P], wv_t[:, t, :],
                start=(t == 0), stop=(t == 1),
            )
        nc.vector.tensor_copy(
            vT_t[:, jt, :, 0:32],
            pv_[:].rearrange("p (h c) -> p h c", h=HEADS),
        )

    # --- attention ---
    for ib in range(NIB):
        isl = slice(ib * IB, (ib + 1) * IB)
        pv_a = ppv.tile([P, IB], F32, tag="pv0")
        pv_b = ppv.tile([P, IB], F32, tag="pv1")
        pv_ps = [pv_a, pv_b]
        for jt in range(NJT):
            jsl = slice(jt * P, (jt + 1) * P)
            if ib == 0:
                # stream the k / vT projections just ahead of first use so
                # the scalar engine starts exp-ing as early as possible
                if jt % 4 == 0:
                    emit_k_chunk(jt * P)
                emit_vt(jt)
            for pair in range(2):
                sim = psim.tile([P, 2, IB], F32, tag="sim")
                for hh in range(2):
                    h = pair * 2 + hh
                    hsl = slice(h * C_HEAD, (h + 1) * C_HEAD)
                    nc.tensor.matmul(
                        sim[:, hh, :], k_t[hsl, jsl], q_t[hsl, isl],
                        start=True, stop=True,
                        tile_position=(h * C_HEAD, 0),
                    )
                e_t = epool.tile([P, 2, IB], BF16, tag="e")
                nc.scalar.activation(e_t, sim, EXP)
                for hh in range(2):
                    h = pair * 2 + hh
                    nc.tensor.matmul(
                        pv_ps[pair][64 * hh : 64 * hh + 33, :],
                        vT_t[:, jt, h, 0:33], e_t[:, hh, :],
                        start=(jt == 0), stop=(jt == NJT - 1),
                        tile_position=(0, 64 * hh),
                    )
        # normalization: head rows scale by 1/denominator per column
        ao = []
        for pair in range(2):
            recip = misc.tile([P, IB], F32, tag=f"recip{pair}")
            nc.vector.reciprocal(recip[0:97, :], pv_ps[pair][0:97, :])
            bc_ps = psim.tile([P, IB], F32, tag="sim")
            for hh in range(2):
                nc.tensor.matmul(
                    bc_ps[64 * hh : 64 * hh + 32, :],
                    ones_row[32 + 64 * hh : 33 + 64 * hh, :],
                    recip[32 + 64 * hh : 33 + 64 * hh, :],
                    start=True, stop=True,
                    tile_position=(32 + 64 * hh, 64 * hh),
                )
            bc_sb = misc.tile([P, IB], F32, tag=f"bcsb{pair}")
            nc.vector.tensor_copy(bc_sb, bc_ps)
            ao_t = misc.tile([P, IB], F32, tag=f"ao{pair}")
            for hh in range(2):
                rs = slice(64 * hh, 64 * hh + 32)
                zs = slice(64 * hh + 32, 64 * hh + 64)
                nc.vector.tensor_mul(ao_t[rs, :], pv_ps[pair][rs, :], bc_sb[rs, :])
                nc.vector.memset(ao_t[zs, :], 0.0)
            ao.append(ao_t)
        # output projection (zero-padded split weights undo the head
        # interleave) + bias
        for ot in range(2):
            pr_ps = ppv.tile([P, IB], F32, tag=f"pv{ot}")
            osl = slice(ot * P, (ot + 1) * P)
            nc.tensor.matmul(pr_ps, woa_t[:, osl], ao[0], start=True, stop=False)
            nc.tensor.matmul(pr_ps, wob_t[:, osl], ao[1], start=False, stop=True)
            o_t = opool.tile([P, IB], F32, tag="o")
            nc.vector.tensor_scalar_add(o_t, pr_ps, bo_t[:, ot : ot + 1])
            nc.sync.dma_start(
                out=out[:].rearrange("(t p) n -> p t n", p=P)[:, ot, isl],
                in_=o_t,
            )


def _get_nc(reps=1):
    key = ("nc", reps)
    if key not in _STATE:
        _STATE[key] = _build_program(reps)
    return _STATE[key]


def _to_bf16(a):
    import ml_dtypes

    return np.ascontiguousarray(a).astype(ml_dtypes.bfloat16)


def _make_in_maps(x, w_qkv, w_out, b_out):
    x = np.ascontiguousarray(x, dtype=np.float32)
    w_qkv = np.asarray(w_qkv, dtype=np.float32)
    w_out = np.asarray(w_out, dtype=np.float32)
    b_out = np.asarray(b_out, dtype=np.float32)
    scale = np.float32(C_HEAD**-0.5)
    wo_t = w_out.T  # [c_hid, c_in]
    woa = np.zeros((C_HID, C_IN), np.float32)
    wob = np.zeros((C_HID, C_IN), np.float32)
    woa[0:32] = wo_t[0:32]  # head 0
    woa[64:96] = wo_t[32:64]  # head 1
    wob[0:32] = wo_t[64:96]  # head 2
    wob[64:96] = wo_t[96:128]  # head 3
    shared = {
        "wq_t": np.ascontiguousarray((w_qkv[0:C_HID] * scale).T),
        "wk_t": np.ascontiguousarray(w_qkv[C_HID : 2 * C_HID].T),
        "wv_t": _to_bf16(w_qkv[2 * C_HID : 3 * C_HID].T),
        "woa_t": woa,
        "wob_t": wob,
        "bo": np.ascontiguousarray(b_out.reshape(2, P)),
    }
    in_maps = []
    for c in range(8):
        b, half = divmod(c, 2)
        xkv = np.ascontiguousarray(x[b].reshape(C_IN, NJ))
        xq = np.ascontiguousarray(xkv[:, half * NI : (half + 1) * NI])
        in_maps.append({"xkv": xkv, "xq": xq, **shared})
    return in_maps


def _assemble(results):
    out = np.empty((B, C_IN, NJ), np.float32)
    for c in range(8):
        b, half = divmod(c, 2)
        out[b][:, half * NI : (half + 1) * NI] = results[c]["out"]
    return out.reshape(B, C_IN, 64, 64)


def _run(in_maps, reps=1, **kwargs):
    from concourse.bass_utils import run_bass_kernel_spmd

    return run_bass_kernel_spmd(
        _get_nc(reps), in_maps, core_ids=list(range(8)), **kwargs
    )


def kernel(x, w_qkv, w_out, b_out):
    res = _run(_make_in_maps(x, w_qkv, w_out, b_out))
    return _assemble(res.results)
